# revision 1
# baseline (speedup 1.0000x reference)
"""Trainium2 Bass kernel for nn_CrossLayer (protein/drug cross-attention).

Reference math (per batch n):
  pg = group_mean(protein, 4)   # (512, 512)
  dg = group_mean(drug, 2)      # (128, 512)
  q/k/v projections (8 heads, dh=64), cross logits, softmax over the
  "other" sequence, attention-weighted values, masked mean-pool over the
  own sequence, concat(prot_embed, drug_embed) -> (1024,)

Key algebraic simplification used on device: the pooled output
  prot_embed_h = sum_l w[l] * (alpha_h @ vd_h)[l, :]
              = (u_h.T @ P_h) @ vd_h          with u_h = w / rowsum_h
so the full (L x L') attention-output einsum is never materialized —
only per-head vectors c_h = P_h.T @ u_h and a tiny c_h.T @ v matvec.

Sharding: data-parallel over batch N=64 across 8 cores (8 batches/core),
weights replicated. All matmuls in bf16 (PE full rate), fp32 PSUM accum.

Schedule (v2): per-batch pipeline with batch-level software pipelining:
iteration b issues DMA loads for b+1, computes grouping/projections/
logits/exp/rowsums for b, then the (cheap) pooled-output matmuls for
b-1 so the PE never waits on the Act/DVE softmax chain. PSUM->SBUF
evacuations are spread across Pool/DVE (Act keeps the exps), softmax
rowsums run in bf16 (DVE 4x mode), and inputs move in a few large DMAs.
"""

import sys

import numpy as np

for _p in ("/opt/trn_rl_repo", "/root/.axon_site/_ro/trn_rl_repo"):
    if _p not in sys.path:
        sys.path.insert(0, _p)

HID = 512
H = 8
DH = 64
GS_P = 4
GS_D = 2
LP_FULL = 2048
LD_FULL = 256
LP = LP_FULL // GS_P  # 512 grouped protein length
LD = LD_FULL // GS_D  # 128 grouped drug length
NB = 64  # total batch
NCORES = 8
B = NB // NCORES  # 8 batches per core
P = 128  # partitions
KT = HID // P  # 4 contraction tiles over hidden dim

_CACHE = {}


def _numpy_reference(protein, drug, mask_prot, mask_drug, Wqp, Wkp, Wvp, Wqd, Wkd, Wvd):
    """Exact reference math in numpy (fallback for non-trivial masks)."""
    INF = 1000000.0

    def group(x, m, gs):
        n, l, d = x.shape
        xg = x.reshape(n, l // gs, gs, d).mean(axis=2)
        mg = m.reshape(n, l // gs, gs).any(axis=2)
        return xg, mg

    def heads(x):
        n, l, d = x.shape
        return x.reshape(n, l, H, d // H)

    pg, mp = group(protein, mask_prot, GS_P)
    dg, md = group(drug, mask_drug, GS_D)
    qp = heads(pg @ Wqp.T)
    kp = heads(pg @ Wkp.T)
    vp = heads(pg @ Wvp.T)
    qd = heads(dg @ Wqd.T)
    kd = heads(dg @ Wkd.T)
    vd = heads(dg @ Wvd.T)

    def alpha(logits, mr, mc):
        pair = mr[:, :, None, None] & mc[:, None, :, None]
        logits = np.where(pair, logits, logits - INF)
        m = logits.max(axis=2, keepdims=True)
        e = np.exp(logits - m)
        a = e / e.sum(axis=2, keepdims=True)
        return np.where(mr[:, :, None, None], a, 0.0)

    lpd = np.einsum("blhd,bkhd->blkh", qp, kd)
    ldp = np.einsum("blhd,bkhd->blkh", qd, kp)
    apd = alpha(lpd, mp, md)
    adp = alpha(ldp, md, mp)
    n = pg.shape[0]
    pe = np.einsum("blkh,bkhd->blhd", apd, vd).reshape(n, pg.shape[1], -1)
    de = np.einsum("blkh,bkhd->blhd", adp, vp).reshape(n, dg.shape[1], -1)
    mpf = mp.astype(pe.dtype)
    mdf = md.astype(de.dtype)
    pemb = (pe * mpf[:, :, None]).sum(axis=1) / mpf.sum(axis=-1)[:, None]
    demb = (de * mdf[:, :, None]).sum(axis=1) / mdf.sum(axis=-1)[:, None]
    return np.concatenate([pemb, demb], axis=1).astype(np.float32)


def _split_excess_waits(nc):
    """Split multi-sem waits into single-wait engine NOPs.

    TPB compute-instruction encodings carry exactly one sync-wait slot
    (NEURON_ISA_TPB_EVENTS); Tile sometimes assigns 2-3 waits to one
    instruction (psum slot tenancy transitions), which walrus rejects with
    "Too many sync wait commands". Since each engine dispatches its stream
    in order, prefixing the instruction with NOPs that each carry one of
    the excess waits is semantically identical.

    DMA instructions are different: their wait condition lives in the DGE
    descriptor and fires autonomously, so a NOP placed before them in the SP
    stream does NOT gate the transfer. For multi-wait DMAs we instead chain
    all waits through SP NOPs that bump a fresh "gate" semaphore, and give
    the descriptor a single gate>=k wait.
    """
    import concourse.mybir as mybir
    import bass_rust

    MULTI_OK = {"InstEventSemaphore"}

    def make_nop(engine):
        eng = {
            mybir.EngineType.PE: nc.tensor,
            mybir.EngineType.Activation: nc.scalar,
            mybir.EngineType.DVE: nc.vector,
            mybir.EngineType.Pool: nc.gpsimd,
            mybir.EngineType.SP: nc.sync,
        }[engine]
        bi = eng.nop(nofuse=True)
        inst = bi.ins if hasattr(bi, "ins") else bi
        # remove from wherever add_instruction appended it
        for bbw in nc.bb_map.values():
            lst = bbw.bb.instructions
            if lst and lst[-1] is inst:
                lst.pop()
                break
        return inst

    # pick a free semaphore id for the DMA gate
    used = set()
    for bbw in nc.bb_map.values():
        for inst in bbw.bb.instructions:
            si = getattr(inst, "sync_info", None)
            if si is None:
                continue
            for w in si.on_wait or []:
                used.add(w.id)
            for u in si.on_update or []:
                used.add(u.id)
    gate_id = max(used) + 1 if used else 100
    assert gate_id < 250, f"no free semaphore for DMA gate ({gate_id})"
    gate_count = 0

    n_split = 0
    for bbw in list(nc.bb_map.values()):
        bb = bbw.bb
        lst = bb.instructions
        idx = 0
        while idx < len(lst):
            inst = lst[idx]
            si = getattr(inst, "sync_info", None)
            if (
                si is not None
                and si.on_wait
                and len(si.on_wait) > 1
                and type(inst).__name__ not in MULTI_OK
            ):
                waits = list(si.on_wait)
                if type(inst).__name__ == "InstDMACopy":
                    # all waits go to SP nops; last nop bumps the gate;
                    # descriptor waits on the gate alone.
                    for w in waits:
                        nop = make_nop(mybir.EngineType.SP)
                        nop.sync_info = type(si)(on_wait=[w], on_update=[])
                        lst.insert(idx, nop)
                        idx += 1
                        n_split += 1
                    gate_count += 1
                    nop.sync_info = type(si)(
                        on_wait=[w],
                        on_update=[
                            bass_rust.SyncUpdate(
                                sync_type="semaphore",
                                id=gate_id,
                                ant_name=f"dma_gate_{gate_id}",
                                update_mode="sem-inc",
                                update_value=1,
                                update_reg=None,
                            )
                        ],
                    )
                    inst.sync_info = type(si)(
                        on_wait=[
                            bass_rust.SyncWait(
                                sync_type="semaphore",
                                id=gate_id,
                                ant_name=f"dma_gate_{gate_id}",
                                wait_mode="sem-ge-imm",
                                wait_value=gate_count,
                                wait_reg=None,
                            )
                        ],
                        on_update=si.on_update,
                    )
                else:
                    extra, keep = waits[:-1], waits[-1:]
                    for w in extra:
                        nop = make_nop(inst.engine)
                        nop.sync_info = type(si)(on_wait=[w], on_update=[])
                        lst.insert(idx, nop)
                        idx += 1
                        n_split += 1
                    inst.sync_info = type(si)(on_wait=keep, on_update=si.on_update)
            idx += 1
    return n_split


def _build_nc():
    import concourse.bass as bass
    import concourse.mybir as mybir
    import concourse.tile as tile

    bf16 = mybir.dt.bfloat16
    f32 = mybir.dt.float32
    fp8 = mybir.dt.float8e4
    DR = mybir.MatmulPerfMode.DoubleRow
    AF = mybir.ActivationFunctionType
    AX = mybir.AxisListType

    nc = bass.Bass()

    prot = nc.declare_dram_parameter("protein", [B, LP_FULL, HID], bf16, isOutput=False)
    drug = nc.declare_dram_parameter("drug", [B, LD_FULL, HID], bf16, isOutput=False)
    # all six weights pre-transposed+tiled on host: [p, 6*KT, HID]
    # where [p, wi*KT+kt, o] = W[wi][o, kt*128+p]
    wall_d = nc.declare_dram_parameter("Wall", [P, 6 * KT, HID], bf16, isOutput=False)
    # grouping matrices, packed: cols [0:32) Gp/4, [32:96) Gd/2
    g_d = nc.declare_dram_parameter("Gboth", [P, 96], bf16, isOutput=False)
    # pooling weights: [b, p, 0:4] = wp[b, t*128+p], [b, p, 4] = wd[b, p]
    wv_d = nc.declare_dram_parameter("wvec", [B, P, 5], f32, isOutput=False)
    out_d = nc.declare_dram_parameter("out", [B, 2 * HID], f32, isOutput=True)

    WI = {"Wqp": 0, "Wkp": 1, "Wvp": 2, "Wqd": 3, "Wkd": 4, "Wvd": 5}
    INV_WS = 0.125  # undo the x8 host-side weight scaling at evacuation

    with tile.TileContext(nc) as tc:
        with (
            tc.tile_pool(name="const", bufs=1) as cpool,
            tc.tile_pool(name="pt", bufs=2) as ptpool,
            tc.tile_pool(name="act", bufs=2) as apool,
            tc.tile_pool(name="pA", bufs=2, space="PSUM") as pA,
            tc.tile_pool(name="pS", bufs=2, space="PSUM") as pS,
            tc.tile_pool(name="pB", bufs=1, space="PSUM") as pB,
            tc.tile_pool(name="pO", bufs=1, space="PSUM") as pO,
        ):
            # ---- constants ----
            g_sb = cpool.tile([P, 96], bf16, tag="g")
            w_sb = cpool.tile([P, 6 * KT, HID], bf16, tag="wall")
            gp_sb = g_sb[:, 0:32]
            gd_sb = g_sb[:, 32:96]

            def wsl(wname, kt, osl=slice(0, HID)):
                return w_sb[:, WI[wname] * KT + kt, osl]

            # per-batch state that iteration b+1's tail still reads
            state = [None] * B
            LOW = dict(reason="bf16/fp8 activations; tolerance is 2e-2")

            def issue_loads(b, first=False):
                pa = ptpool.tile([P, 8, HID], bf16, tag="pa")
                pb = ptpool.tile([P, 8, HID], bf16, tag="pb")
                dr = ptpool.tile([P, 2, HID], bf16, tag="dr")
                wv = ptpool.tile([P, 5], f32, tag="wv")
                prot_r = prot[b].rearrange("(t p) d -> p t d", p=P)
                if first:
                    nc.sync.dma_start(out=g_sb, in_=g_d[:, :])
                nc.sync.dma_start(out=wv, in_=wv_d[b])
                nc.sync.dma_start(out=pa, in_=prot_r[:, 0:8, :])
                if first:
                    # weights interleave with batch-0 inputs so grouping and
                    # the first projections both start as early as possible
                    nc.sync.dma_start(
                        out=w_sb[:, 0:12, :], in_=wall_d[:, 0:12, :]
                    )
                nc.sync.dma_start(out=pb, in_=prot_r[:, 8:16, :])
                if first:
                    nc.sync.dma_start(
                        out=w_sb[:, 12:24, :], in_=wall_d[:, 12:24, :]
                    )
                nc.sync.dma_start(
                    out=dr, in_=drug[b].rearrange("(t p) d -> p t d", p=P)
                )
                return pa, pb, dr, wv

            def compute(b, loads, mid=None):
                pa, pb, dr, wv = loads
                ACT, VEC = nc.scalar.copy, nc.vector.tensor_copy

                # ---- grouping: pgT [d, lp], kt-pairs share a 2-bank tile ----
                pgT = apool.tile([P, KT, LP], bf16, tag="pgT")
                for ktp in range(2):
                    ps = pA.tile([P, 2 * LP], f32, tag="A")
                    for kt in (2 * ktp, 2 * ktp + 1):
                        for t in range(16):
                            src = pa if t < 8 else pb
                            nc.tensor.matmul(
                                ps[:, (kt % 2) * LP + t * 32 : (kt % 2) * LP + t * 32 + 32],
                                lhsT=src[:, t % 8, kt * P : (kt + 1) * P],
                                rhs=gp_sb,
                                start=True,
                                stop=True,
                            )
                    out_ap = pgT[:, 2 * ktp : 2 * ktp + 2, :].rearrange(
                        "p a b -> p (a b)"
                    )
                    with nc.allow_low_precision(**LOW):
                        if ktp == 0:
                            nc.scalar.copy(out=out_ap, in_=ps)
                        else:
                            nc.vector.tensor_copy(out=out_ap, in_=ps)
                # dgT [d, ld]: one 1-bank tile
                dgT = apool.tile([P, KT, LD], bf16, tag="dgT")
                ps = pB.tile([P, LP], f32, tag="B")
                for kt in range(KT):
                    for t in range(2):
                        nc.tensor.matmul(
                            ps[:, kt * LD + t * 64 : kt * LD + (t + 1) * 64],
                            lhsT=dr[:, t, kt * P : (kt + 1) * P],
                            rhs=gd_sb,
                            start=True,
                            stop=True,
                        )
                with nc.allow_low_precision(**LOW):
                    nc.scalar.copy(out=dgT.rearrange("p a b -> p (a b)"), in_=ps)

                # ---- projections ----
                def proj_lp(wname, tag):
                    # [o, lp] layout, mt-pairs in 2-bank psum tiles;
                    # alternate evac engines so neither gates the psum ring
                    dst = apool.tile([P, KT, LP], bf16, tag=tag)
                    for mtp in range(2):
                        ps = pA.tile([P, 2 * LP], f32, tag="A")
                        for mt in (2 * mtp, 2 * mtp + 1):
                            for kt in range(KT):
                                nc.tensor.matmul(
                                    ps[:, (mt % 2) * LP : (mt % 2) * LP + LP],
                                    lhsT=wsl(wname, kt, slice(mt * P, (mt + 1) * P)),
                                    rhs=pgT[:, kt, :],
                                    start=(kt == 0),
                                    stop=(kt == KT - 1),
                                )
                        out_ap = dst[:, 2 * mtp : 2 * mtp + 2, :].rearrange(
                            "p a b -> p (a b)"
                        )
                        with nc.allow_low_precision(**LOW):
                            if mtp == 0:
                                nc.scalar.copy(out=out_ap, in_=ps)
                            else:
                                nc.vector.tensor_copy(out=out_ap, in_=ps)
                    return dst

                qpT = proj_lp("Wqp", "qpT")
                kpT = proj_lp("Wkp", "kpT")

                # qdT+kdT share one 2-bank tile: [o, ld] each 512 wide
                qkdT = apool.tile([P, 2, KT, LD], bf16, tag="qkdT")
                ps = pA.tile([P, 2 * LP], f32, tag="A")
                for qk, wname in enumerate(["Wqd", "Wkd"]):
                    for mt in range(KT):
                        for kt in range(KT):
                            nc.tensor.matmul(
                                ps[:, qk * LP + mt * LD : qk * LP + (mt + 1) * LD],
                                lhsT=wsl(wname, kt, slice(mt * P, (mt + 1) * P)),
                                rhs=dgT[:, kt, :],
                                start=(kt == 0),
                                stop=(kt == KT - 1),
                            )
                with nc.allow_low_precision(**LOW):
                    nc.scalar.copy(
                        out=qkdT.rearrange("p a b c -> p (a b c)"), in_=ps
                    )

                def head_slice(tens, h):
                    # [o, l]-layout tile [128, KT, len]: head h rows
                    return tens[64 * (h % 2) : 64 * (h % 2) + 64, h // 2, :]

                def head_slice_qkd(qk, h):
                    return qkdT[64 * (h % 2) : 64 * (h % 2) + 64, qk, h // 2, :]

                # vp natural [lp, o] (mt-pairs), vd natural [ld, o]
                vp = apool.tile([P, KT, HID], bf16, tag="vp")
                vd = apool.tile([P, HID], bf16, tag="vd")

                def vp_pair(mtp):
                    ps = pA.tile([P, 2 * LP], f32, tag="A")
                    for mt in (2 * mtp, 2 * mtp + 1):
                        for kt in range(KT):
                            nc.tensor.matmul(
                                ps[:, (mt % 2) * LP : (mt % 2) * LP + LP],
                                lhsT=pgT[:, kt, mt * P : (mt + 1) * P],
                                rhs=wsl("Wvp", kt),
                                start=(kt == 0),
                                stop=(kt == KT - 1),
                            )
                    with nc.allow_low_precision(**LOW):
                        nc.vector.tensor_copy(
                            out=vp[:, 2 * mtp : 2 * mtp + 2, :].rearrange(
                                "p a b -> p (a b)"
                            ),
                            in_=ps,
                        )

                def vd_calc():
                    ps = pB.tile([P, LP], f32, tag="B")
                    for kt in range(KT):
                        nc.tensor.matmul(
                            ps,
                            lhsT=dgT[:, kt, :],
                            rhs=wsl("Wvd", kt),
                            start=(kt == 0),
                            stop=(kt == KT - 1),
                        )
                    with nc.allow_low_precision(**LOW):
                        nc.vector.tensor_copy(out=vd, in_=ps)

                # ---- protein->drug attention: E [lp, (h, ld)] = exp(qp.kd) ----
                # two parity banks inside one 2-bank tile -> single exp, with
                # Pool pre-halving the rowsum reduction (SBUF-only on gpsimd)
                E = apool.tile([P, LP // P, H, LD], bf16, tag="E")
                Eh = apool.tile([P, LP // P, H, LD // 2], bf16, tag="Eh")
                Eq = apool.tile([P, LP // P, H, LD // 4], bf16, tag="Eq")
                rs_pd = apool.tile([P, LP // P, H], bf16, tag="rs_pd")

                def s_pd(lt):
                    # even/odd heads use PE row-groups 0/64; concurrent
                    # row-group matmuls into the same PSUM bank fault the HW,
                    # so each parity gets its own 1-bank tile
                    for par in range(2):
                        ps = pS.tile([P, LP], f32, tag="S")
                        for hh in range(4):
                            h = 2 * hh + par
                            nc.tensor.matmul(
                                ps[:, hh * LD : (hh + 1) * LD],
                                lhsT=head_slice(qpT, h)[:, lt * P : (lt + 1) * P],
                                rhs=head_slice(kdT_, h),
                                start=True,
                                stop=True,
                            )
                        nc.scalar.activation(
                            out=E[:, lt, par : H : 2, :], in_=ps, func=AF.Exp
                        )
                    with nc.allow_low_precision(**LOW):
                        nc.gpsimd.tensor_tensor(
                            out=Eh[:, lt],
                            in0=E[:, lt, :, 0 : LD // 2],
                            in1=E[:, lt, :, LD // 2 : LD],
                            op=mybir.AluOpType.add,
                        )
                        nc.gpsimd.tensor_tensor(
                            out=Eq[:, lt],
                            in0=Eh[:, lt, :, 0 : LD // 4],
                            in1=Eh[:, lt, :, LD // 4 : LD // 2],
                            op=mybir.AluOpType.add,
                        )
                        nc.vector.reduce_sum(
                            out=rs_pd[:, lt, :], in_=Eq[:, lt], axis=AX.X
                        )

                # ---- drug->protein attention: Pdp [ld, (h, lp)] ----
                # rowsums fall out of the exp for free via the Act accumulator
                Pdp = apool.tile([P, H, LP], bf16, tag="Pdp")
                rs_dp = apool.tile([P, H], f32, tag="rs_dp")

                def s_dp(i):
                    # head pair (2i, 2i+1): separate banks + row groups
                    for par in range(2):
                        h = 2 * i + par
                        ps = pS.tile([P, LP], f32, tag="S")
                        nc.tensor.matmul(
                            ps,
                            lhsT=head_slice_qkd(0, h),
                            rhs=head_slice(kpT, h),
                            start=True,
                            stop=True,
                        )
                        nc.scalar.activation(
                            out=Pdp[:, h, :],
                            in_=ps,
                            func=AF.Exp,
                            accum_out=rs_dp[:, h : h + 1],
                        )

                # interleave the S phases with the remaining projections so
                # the Act exp drain keeps pace with PE psum production
                kdT_ = qkdT[:, 1]  # alias with head_slice-compatible layout
                s_pd(0)
                vp_pair(0)
                s_pd(1)
                vp_pair(1)
                s_pd(2)
                vd_calc()
                s_pd(3)
                s_dp(0)
                s_dp(1)
                if mid is not None:
                    mid()
                s_dp(2)
                s_dp(3)

                # ---- u = w / rowsum (bf16) ----
                u_pd = apool.tile([P, LP // P, H], bf16, tag="u_pd")
                inv = apool.tile([P, LP // P, H], f32, tag="inv_pd")
                nc.vector.reciprocal(
                    out=inv.rearrange("p a b -> p (a b)"),
                    in_=rs_pd.rearrange("p a b -> p (a b)"),
                )
                for lt in range(LP // P):
                    nc.vector.tensor_scalar_mul(
                        u_pd[:, lt, :], inv[:, lt, :], wv[:, lt : lt + 1]
                    )
                u_dp = apool.tile([P, H], bf16, tag="u_dp")
                inv2 = apool.tile([P, H], f32, tag="inv_dp")
                nc.vector.reciprocal(out=inv2, in_=rs_dp)
                nc.vector.tensor_scalar_mul(u_dp, inv2, wv[:, 4:5])

                state[b] = (E, Pdp, u_pd, u_dp, vp, vd)

            def finish(b):
                E, Pdp, u_pd, u_dp, vp, vd = state[b]
                # c_pd [ld, h] = sum_lp E[lp, h, ld] * u_pd[lp, h]
                # c_dp [lp-sub, (lt, h)] = sum_ld Pdp[ld, h, lp] * u_dp[ld, h]
                # one psum bank: cols [0:8) c_pd, [8:40) c_dp
                ps_c = pB.tile([P, LP], f32, tag="B")
                for h in range(H):
                    for lt in range(LP // P):
                        nc.tensor.matmul(
                            ps_c[:, h : h + 1],
                            lhsT=E[:, lt, h, :],
                            rhs=u_pd[:, lt, h : h + 1],
                            start=(lt == 0),
                            stop=(lt == LP // P - 1),
                        )
                for h in range(H):
                    for lt in range(LP // P):
                        nc.tensor.matmul(
                            ps_c[:, H + lt * H + h : H + lt * H + h + 1],
                            lhsT=Pdp[:, h, lt * P : (lt + 1) * P],
                            rhs=u_dp[:, h : h + 1],
                            start=True,
                            stop=True,
                        )
                c_all = apool.tile([P, H + KT * H], bf16, tag="c_all")
                with nc.allow_low_precision(**LOW):
                    nc.vector.tensor_copy(out=c_all, in_=ps_c[:, 0 : H + KT * H])
                c_pd = c_all[:, 0:H]
                c_dp = c_all[:, H:].rearrange("p (a b) -> p a b", a=KT)

                # ---- final embeddings (DMA'd to DRAM straight from psum) ----
                ps_o1 = pO.tile([1, HID], f32, tag="O")
                for h in range(H):
                    nc.tensor.matmul(
                        ps_o1[:, h * DH : (h + 1) * DH],
                        lhsT=c_pd[:, h : h + 1],
                        rhs=vd[:, h * DH : (h + 1) * DH],
                        start=True,
                        stop=True,
                    )
                outst = apool.tile([1, 2 * HID], f32, tag="outst")
                nc.scalar.copy(out=outst[:, 0:HID], in_=ps_o1)
                ps_o2 = pO.tile([1, HID], f32, tag="O")
                for h in range(H):
                    for lt in range(LP // P):
                        nc.tensor.matmul(
                            ps_o2[:, h * DH : (h + 1) * DH],
                            lhsT=c_dp[:, lt, h : h + 1],
                            rhs=vp[:, lt, h * DH : (h + 1) * DH],
                            start=(lt == 0),
                            stop=(lt == LP // P - 1),
                        )
                nc.vector.tensor_copy(out=outst[:, HID : 2 * HID], in_=ps_o2)
                nc.sync.dma_start(out=out_d[b : b + 1, :], in_=outst)

            loads = issue_loads(0, first=True)
            for b in range(B):
                nxt = issue_loads(b + 1) if b + 1 < B else None
                mid = (lambda bb: lambda: finish(bb))(b - 1) if b > 0 else None
                compute(b, loads, mid=mid)
                loads = nxt
            finish(B - 1)

    _split_excess_waits(nc)
    return nc


def _prep_in_maps(inputs):
    """Returns (in_maps, None) for the device path, or (None, fallback_out)."""
    protein = np.asarray(inputs["protein"], dtype=np.float32)
    drug = np.asarray(inputs["drug"], dtype=np.float32)
    mask_prot = np.asarray(inputs["mask_prot"]).astype(bool)
    mask_drug = np.asarray(inputs["mask_drug"]).astype(bool)
    Ws = {w: np.asarray(inputs[w], dtype=np.float32) for w in
          ["Wqp", "Wkp", "Wvp", "Wqd", "Wkd", "Wvd"]}

    import ml_dtypes

    bf = ml_dtypes.bfloat16

    # Grouped masks / pooling weights (general in the pooling path).
    mp = mask_prot.reshape(NB, LP, GS_P).any(axis=2)
    md = mask_drug.reshape(NB, LD, GS_D).any(axis=2)
    if not (mp.all() and md.all()):
        # Masked-out grouped positions change the softmax column masking —
        # handled exactly on the host (inputs per spec are all-ones).
        return None, _numpy_reference(
            protein, drug, mask_prot, mask_drug,
            Ws["Wqp"], Ws["Wkp"], Ws["Wvp"], Ws["Wqd"], Ws["Wkd"], Ws["Wvd"],
        )
    wp = (mp.astype(np.float32) / mp.sum(axis=1, keepdims=True)).astype(np.float32)
    wd = (md.astype(np.float32) / md.sum(axis=1, keepdims=True)).astype(np.float32)

    # Host-side layout prep (cheap): bf16 casts + weight transposes.
    prot_bf = protein.astype(bf)
    drug_bf = drug.astype(bf)
    # Wall[p, wi*KT+kt, o] = W[wi][o, kt*128+p]
    wall = np.empty((P, 6 * KT, HID), dtype=bf)
    for wi, w in enumerate(["Wqp", "Wkp", "Wvp", "Wqd", "Wkd", "Wvd"]):
        wT = Ws[w].T.astype(bf)  # [d, o]
        wall[:, wi * KT : (wi + 1) * KT, :] = wT.reshape(KT, P, HID).transpose(1, 0, 2)
    gboth = np.zeros((P, 96), dtype=bf)
    for g in range(P // GS_P):
        gboth[GS_P * g : GS_P * (g + 1), g] = 1.0 / GS_P
    for g in range(P // GS_D):
        gboth[GS_D * g : GS_D * (g + 1), 32 + g] = 1.0 / GS_D

    # wvec[b, p, 0:4] = wp[b, t*128+p]; wvec[b, p, 4] = wd[b, p]
    wvec = np.empty((NB, P, 5), dtype=np.float32)
    wvec[:, :, 0:4] = wp.reshape(NB, KT, P).transpose(0, 2, 1)
    wvec[:, :, 4] = wd

    in_maps = []
    for c in range(NCORES):
        sl = slice(c * B, (c + 1) * B)
        in_maps.append(
            {
                "protein": np.ascontiguousarray(prot_bf[sl]),
                "drug": np.ascontiguousarray(drug_bf[sl]),
                "Wall": wall,
                "Gboth": gboth,
                "wvec": np.ascontiguousarray(wvec[sl]),
            }
        )
    return in_maps, None


def kernel(**inputs):
    in_maps, fallback = _prep_in_maps(inputs)
    if in_maps is None:
        return fallback

    if "nc" not in _CACHE:
        _CACHE["nc"] = _build_nc()
    nc = _CACHE["nc"]

    from concourse.bass_utils import run_bass_kernel_spmd

    res = run_bass_kernel_spmd(nc, in_maps, list(range(NCORES)))
    _CACHE["last_results"] = res
    out = np.concatenate([res.results[c]["out"] for c in range(NCORES)], axis=0)
    return out.astype(np.float32)


if __name__ == "__main__":
    rng = np.random.default_rng(0)
    inputs = {
        "protein": rng.standard_normal((NB, LP_FULL, HID), dtype=np.float32),
        "drug": rng.standard_normal((NB, LD_FULL, HID), dtype=np.float32),
        "mask_prot": np.ones((NB, LP_FULL), dtype=bool),
        "mask_drug": np.ones((NB, LD_FULL), dtype=bool),
    }
    for w in ["Wqp", "Wkp", "Wvp", "Wqd", "Wkd", "Wvd"]:
        inputs[w] = rng.standard_normal((HID, HID), dtype=np.float32) / np.sqrt(HID)
    out = kernel(**inputs)
    ref = _numpy_reference(
        inputs["protein"], inputs["drug"], inputs["mask_prot"], inputs["mask_drug"],
        inputs["Wqp"], inputs["Wkp"], inputs["Wvp"],
        inputs["Wqd"], inputs["Wkd"], inputs["Wvd"],
    )
    err = np.abs(out - ref).max() / np.abs(ref).max()
    print("rel err:", err)



# revision 21
# speedup vs baseline: 1.2196x; 1.2196x over previous
"""Trainium2 Bass kernel for nn_CrossLayer (protein/drug cross-attention).

Reference math (per batch n):
  pg = group_mean(protein, 4)   # (512, 512)
  dg = group_mean(drug, 2)      # (128, 512)
  q/k/v projections (8 heads, dh=64), cross logits, softmax over the
  "other" sequence, attention-weighted values, masked mean-pool over the
  own sequence, concat(prot_embed, drug_embed) -> (1024,)

Key algebraic simplification: the pooled output
  prot_embed_h = (u_h.T @ P_h) @ vd_h   with u_h = w / rowsum_h
so the full (L x L') attention-output einsum is never materialized.

v3 scheme:
- All six projections run as fp8e4 DoubleRow matmuls with 3-term
  error compensation: x@W ~= xh@Wh + (xl@Wh + xh@Wl), where xh/xl are
  the fp8 hi/lo split of 64*x (scale keeps the lo part out of fp8
  subnormals). 6 DR instructions per 128-row output tile instead of 8
  bf16 instructions -> 25% fewer PE cycles, accuracy ~2x better than
  bf16.
- Scales ride the psum: q/k/v tensors are stored as 4096*value; the
  softmax descales via exp(scale=2^-24); outputs descale at the final
  copy (2^-12). All other PSUM evacuations are plain copies.
- Grouping is pipelined one batch ahead so its evac latency hides
  behind the previous batch's projections.
- Softmax: one 2-bank PSUM tile + one exp per 4 logit column-groups;
  rowsums via single DVE 4x-mode reduces; no Pool (no PSUM port).

Sharding: data-parallel over batch N=64 across 8 cores, weights
replicated.
"""

import sys

import numpy as np

for _p in ("/opt/trn_rl_repo", "/root/.axon_site/_ro/trn_rl_repo"):
    if _p not in sys.path:
        sys.path.insert(0, _p)

HID = 512
H = 8
DH = 64
GS_P = 4
GS_D = 2
LP_FULL = 2048
LD_FULL = 256
LP = LP_FULL // GS_P  # 512 grouped protein length
LD = LD_FULL // GS_D  # 128 grouped drug length
NB = 64  # total batch
NCORES = 8
B = NB // NCORES  # 8 batches per core
P = 128  # partitions
KT = HID // P  # 4 contraction tiles over hidden dim
SC = 64.0  # fp8 hi/lo split scale (pg and W each carry 64x)

_CACHE = {}
PHASE_MARKS = []  # (label, first_I_number) appended during _build_nc


def _mark(nc, label):
    nm = nc.get_next_instruction_name()  # consumes one name: I-<n>
    PHASE_MARKS.append((label, int(nm.split("-")[1])))


def _numpy_reference(protein, drug, mask_prot, mask_drug, Wqp, Wkp, Wvp, Wqd, Wkd, Wvd):
    """Exact reference math in numpy (fallback for non-trivial masks)."""
    INF = 1000000.0

    def group(x, m, gs):
        n, l, d = x.shape
        xg = x.reshape(n, l // gs, gs, d).mean(axis=2)
        mg = m.reshape(n, l // gs, gs).any(axis=2)
        return xg, mg

    def heads(x):
        n, l, d = x.shape
        return x.reshape(n, l, H, d // H)

    pg, mp = group(protein, mask_prot, GS_P)
    dg, md = group(drug, mask_drug, GS_D)
    qp = heads(pg @ Wqp.T)
    kp = heads(pg @ Wkp.T)
    vp = heads(pg @ Wvp.T)
    qd = heads(dg @ Wqd.T)
    kd = heads(dg @ Wkd.T)
    vd = heads(dg @ Wvd.T)

    def alpha(logits, mr, mc):
        pair = mr[:, :, None, None] & mc[:, None, :, None]
        logits = np.where(pair, logits, logits - INF)
        m = logits.max(axis=2, keepdims=True)
        e = np.exp(logits - m)
        a = e / e.sum(axis=2, keepdims=True)
        return np.where(mr[:, :, None, None], a, 0.0)

    lpd = np.einsum("blhd,bkhd->blkh", qp, kd)
    ldp = np.einsum("blhd,bkhd->blkh", qd, kp)
    apd = alpha(lpd, mp, md)
    adp = alpha(ldp, md, mp)
    n = pg.shape[0]
    pe = np.einsum("blkh,bkhd->blhd", apd, vd).reshape(n, pg.shape[1], -1)
    de = np.einsum("blkh,bkhd->blhd", adp, vp).reshape(n, dg.shape[1], -1)
    mpf = mp.astype(pe.dtype)
    mdf = md.astype(de.dtype)
    pemb = (pe * mpf[:, :, None]).sum(axis=1) / mpf.sum(axis=-1)[:, None]
    demb = (de * mdf[:, :, None]).sum(axis=1) / mdf.sum(axis=-1)[:, None]
    return np.concatenate([pemb, demb], axis=1).astype(np.float32)


def _split_excess_waits(nc):
    """Split multi-sem waits into single-wait engine NOPs.

    TPB compute-instruction encodings carry exactly one sync-wait slot;
    Tile sometimes assigns 2-3 waits to one instruction, which walrus
    rejects. Prefixing the instruction with NOPs that each carry one of
    the excess waits is semantically identical (engines dispatch their
    stream in order). DMA waits instead chain through SP NOPs bumping a
    gate semaphore (DGE wait conditions fire autonomously).
    """
    import concourse.mybir as mybir
    import bass_rust

    MULTI_OK = {"InstEventSemaphore"}

    def make_nop(engine):
        eng = {
            mybir.EngineType.PE: nc.tensor,
            mybir.EngineType.Activation: nc.scalar,
            mybir.EngineType.DVE: nc.vector,
            mybir.EngineType.Pool: nc.gpsimd,
            mybir.EngineType.SP: nc.sync,
        }[engine]
        bi = eng.nop(nofuse=True)
        inst = bi.ins if hasattr(bi, "ins") else bi
        for bbw in nc.bb_map.values():
            lst = bbw.bb.instructions
            if lst and lst[-1] is inst:
                lst.pop()
                break
        return inst

    used = set()
    for bbw in nc.bb_map.values():
        for inst in bbw.bb.instructions:
            si = getattr(inst, "sync_info", None)
            if si is None:
                continue
            for w in si.on_wait or []:
                used.add(w.id)
            for u in si.on_update or []:
                used.add(u.id)
    gate_id = max(used) + 1 if used else 100
    assert gate_id < 250, f"no free semaphore for DMA gate ({gate_id})"
    gate_count = 0

    n_split = 0
    for bbw in list(nc.bb_map.values()):
        bb = bbw.bb
        lst = bb.instructions
        idx = 0
        while idx < len(lst):
            inst = lst[idx]
            si = getattr(inst, "sync_info", None)
            if (
                si is not None
                and si.on_wait
                and len(si.on_wait) > 1
                and type(inst).__name__ not in MULTI_OK
            ):
                waits = list(si.on_wait)
                if type(inst).__name__ == "InstDMACopy":
                    for w in waits:
                        nop = make_nop(mybir.EngineType.SP)
                        nop.sync_info = type(si)(on_wait=[w], on_update=[])
                        lst.insert(idx, nop)
                        idx += 1
                        n_split += 1
                    gate_count += 1
                    nop.sync_info = type(si)(
                        on_wait=[w],
                        on_update=[
                            bass_rust.SyncUpdate(
                                sync_type="semaphore",
                                id=gate_id,
                                ant_name=f"dma_gate_{gate_id}",
                                update_mode="sem-inc",
                                update_value=1,
                                update_reg=None,
                            )
                        ],
                    )
                    inst.sync_info = type(si)(
                        on_wait=[
                            bass_rust.SyncWait(
                                sync_type="semaphore",
                                id=gate_id,
                                ant_name=f"dma_gate_{gate_id}",
                                wait_mode="sem-ge-imm",
                                wait_value=gate_count,
                                wait_reg=None,
                            )
                        ],
                        on_update=si.on_update,
                    )
                else:
                    extra, keep = waits[:-1], waits[-1:]
                    for w in extra:
                        nop = make_nop(inst.engine)
                        nop.sync_info = type(si)(on_wait=[w], on_update=[])
                        lst.insert(idx, nop)
                        idx += 1
                        n_split += 1
                    inst.sync_info = type(si)(on_wait=keep, on_update=si.on_update)
            idx += 1
    return n_split


def _build_nc():
    import concourse.bass as bass
    import concourse.mybir as mybir
    import concourse.tile as tile

    bf16 = mybir.dt.bfloat16
    f32 = mybir.dt.float32
    fp8 = mybir.dt.float8e4
    DR = mybir.MatmulPerfMode.DoubleRow
    AF = mybir.ActivationFunctionType
    AX = mybir.AxisListType
    SUB = mybir.AluOpType.subtract

    nc = bass.Bass()

    prot = nc.declare_dram_parameter("protein", [B, LP_FULL, HID], bf16, isOutput=False)
    drug = nc.declare_dram_parameter("drug", [B, LD_FULL, HID], bf16, isOutput=False)
    # fp8 hi/lo split weights: rows r = wi*2*KT + kt*2 + s (s: 0=hi, 1=lo)
    # value = split_s(SC * W[wi][o, kt*128+p])
    w_d = nc.declare_dram_parameter("Wall8", [P, 6 * 2 * KT, HID], fp8, isOutput=False)
    # grouping matrices: cols [0:32) Gp*(SC/4), [32:96) Gd*(SC/2)
    g_d = nc.declare_dram_parameter("Gboth", [P, 96], bf16, isOutput=False)
    # pooling weights: [b, p, 0:4] = wp[b, t*128+p], [b, p, 4] = wd[b, p]
    wv_d = nc.declare_dram_parameter("wvec", [B, P, 5], f32, isOutput=False)
    out_d = nc.declare_dram_parameter("out", [B, 2 * HID], f32, isOutput=True)

    WI = {"Wqp": 0, "Wkp": 1, "Wvp": 2, "Wqd": 3, "Wkd": 4, "Wvd": 5}
    EXP_SCALE = 1.0 / (SC * SC * SC * SC)  # 2^-24: descale logits at exp
    OUT_SCALE = 1.0 / (SC * SC)  # 2^-12: descale v at output evac

    with tile.TileContext(nc) as tc:
        with (
            tc.tile_pool(name="const", bufs=1) as cpool,
            tc.tile_pool(name="pt", bufs=2) as ptpool,
            tc.tile_pool(name="act", bufs=2) as apool,
            tc.tile_pool(name="pP", bufs=2, space="PSUM") as pP,
            tc.tile_pool(name="pS", bufs=2, space="PSUM") as pS,
        ):
            g_sb = cpool.tile([P, 96], bf16, tag="g")
            w_sb = cpool.tile([P, 6 * 2 * KT, HID], fp8, tag="wall")
            gp_sb = g_sb[:, 0:32]
            gd_sb = g_sb[:, 32:96]

            def w8(wname, row, osl=slice(0, HID)):
                """Single [128, o] row of the hi/lo weight stack."""
                return w_sb[:, WI[wname] * 2 * KT + row, osl]

            def w8p(wname, r0, step, osl=slice(0, HID)):
                """[128, 2, o] row-pair (the two DR groups)."""
                base = WI[wname] * 2 * KT
                return w_sb[:, base + r0 : base + r0 + step + 1 : step, osl]

            state = [None] * B
            # pgboth rows: kt*2+0 = lo, kt*2+1 = hi (pairs with Wall8's hi,lo)
            grp = [None] * B  # (pgboth, dgboth, wv) per batch
            LOW = dict(reason="bf16/fp8 activations; tolerance is 2e-2")

            def issue_loads(b, first=False):
                pa = ptpool.tile([P, 8, HID], bf16, tag="pa")
                pb = ptpool.tile([P, 8, HID], bf16, tag="pb")
                dr = ptpool.tile([P, 2, HID], bf16, tag="dr")
                wv = ptpool.tile([P, 5], f32, tag="wv")
                prot_r = prot[b].rearrange("(t p) d -> p t d", p=P)
                if first:
                    nc.sync.dma_start(out=g_sb, in_=g_d[:, :])
                nc.sync.dma_start(out=wv, in_=wv_d[b])
                nc.sync.dma_start(out=pa, in_=prot_r[:, 0:8, :])
                nc.sync.dma_start(out=pb, in_=prot_r[:, 8:16, :])
                nc.sync.dma_start(
                    out=dr, in_=drug[b].rearrange("(t p) d -> p t d", p=P)
                )
                if first:
                    # weights are first needed by the projections, after
                    # grouping(0) -- load them behind batch-0 inputs
                    nc.sync.dma_start(out=w_sb[:, 0:24, :], in_=w_d[:, 0:24, :])
                    nc.sync.dma_start(out=w_sb[:, 24:48, :], in_=w_d[:, 24:48, :])
                return pa, pb, dr, wv

            def grouping(b, loads):
                """Fill pgboth/dgboth (fp8 hi/lo, values SC*pg) for batch b."""
                _mark(nc, f"grouping({b})")
                pa, pb, dr, wv = loads
                pgboth = apool.tile([P, 2 * KT, LP], fp8, tag="pgboth")
                dgboth = apool.tile([P, 2 * KT, LD], fp8, tag="dgboth")
                for ktp in range(2):
                    ps = pP.tile([P, 2 * LP], f32, tag="P")
                    for t in range(16):
                        src = pa if t < 8 else pb
                        for kt in (2 * ktp, 2 * ktp + 1):
                            nc.tensor.matmul(
                                ps[:, (kt % 2) * LP + t * 32 : (kt % 2) * LP + t * 32 + 32],
                                lhsT=src[:, t % 8, kt * P : (kt + 1) * P],
                                rhs=gp_sb,
                                start=True,
                                stop=True,
                            )
                    # hi rows (4ktp+1, 4ktp+3), lo rows (4ktp, 4ktp+2)
                    hi = pgboth[:, 4 * ktp + 1 : 4 * ktp + 4 : 2, :]
                    lo = pgboth[:, 4 * ktp : 4 * ktp + 3 : 2, :]
                    ps3 = ps.rearrange("p (a b) -> p a b", a=2)
                    with nc.allow_low_precision(**LOW):
                        nc.vector.tensor_copy(out=hi, in_=ps3)
                        nc.vector.tensor_tensor(
                            out=lo, in0=ps3, in1=hi, op=SUB
                        )
                ps = pP.tile([P, 2 * LP], f32, tag="P")
                psd = ps[:, 0 : KT * LD]
                for kt in range(KT):
                    for t in range(2):
                        nc.tensor.matmul(
                            psd[:, kt * LD + t * 64 : kt * LD + (t + 1) * 64],
                            lhsT=dr[:, t, kt * P : (kt + 1) * P],
                            rhs=gd_sb,
                            start=True,
                            stop=True,
                        )
                psd3 = psd.rearrange("p (a b) -> p a b", a=KT)
                hi = dgboth[:, 1 : 2 * KT : 2, :]
                lo = dgboth[:, 0 : 2 * KT - 1 : 2, :]
                with nc.allow_low_precision(**LOW):
                    nc.vector.tensor_copy(out=hi, in_=psd3)
                    nc.vector.tensor_tensor(out=lo, in0=psd3, in1=hi, op=SUB)
                grp[b] = (pgboth, dgboth, wv)

            def dr_proj(ps_out, wname, both, lp_sl=None, w_sl=None, wlhs=True):
                """3-term compensated DR projection into ps_out.

                wlhs=True: lhsT = weight rows, rhs = activation rows
                           (out = [o_tile, l]).
                wlhs=False: lhsT = activation rows, rhs = weight rows
                           (out = [l_tile, o]).
                both rows: kt*2+0 = lo, kt*2+1 = hi.
                """
                n = 0

                def mm(wpair, apair, last):
                    nonlocal n
                    lhsT, rhs = (wpair, apair) if wlhs else (apair, wpair)
                    nc.tensor.matmul(
                        ps_out,
                        lhsT=lhsT,
                        rhs=rhs,
                        start=(n == 0),
                        stop=last,
                        perf_mode=DR,
                    )
                    n += 1

                asl = lp_sl if lp_sl is not None else slice(None)
                # HI terms: W rows (4i, 4i+2) [hi pair], act rows (4i+1, 4i+3)
                for i in range(2):
                    mm(
                        w8p(wname, 4 * i, 2, w_sl) if w_sl else w8p(wname, 4 * i, 2),
                        both[:, 4 * i + 1 : 4 * i + 4 : 2, asl],
                        False,
                    )
                # CORR: W rows (2k, 2k+1) = (hi_k, lo_k), act rows (2k, 2k+1)
                # = (lo_k, hi_k) -> lo@Whi + hi@Wlo
                for k in range(KT):
                    mm(
                        w8p(wname, 2 * k, 1, w_sl) if w_sl else w8p(wname, 2 * k, 1),
                        both[:, 2 * k : 2 * k + 2, asl],
                        k == KT - 1,
                    )

            def head_slice(tens, h):
                return tens[64 * (h % 2) : 64 * (h % 2) + 64, h // 2, :]

            def head_slice_qkd(qk, h):
                return state_qkdT[64 * (h % 2) : 64 * (h % 2) + 64, qk, h // 2, :]

            state_qkdT = None

            def compute(b, mid_pd=None, mid_dp=None, tail=None, grp_next=None):
                nonlocal state_qkdT
                pgboth, dgboth, wv = grp[b]

                # ---- qpT, kpT [o, lp] (scaled 4096x), via mt-pair psum ----
                qpT = apool.tile([P, KT, LP], bf16, tag="qpT")
                kpT = apool.tile([P, KT, LP], bf16, tag="kpT")
                qkdT = apool.tile([P, 2, KT, LD], bf16, tag="qkdT")
                state_qkdT = qkdT

                def proj_lp(wname, dst, evac):
                    _mark(nc, f"proj_{wname}({b})")
                    for mtp in range(2):
                        ps = pP.tile([P, 2 * LP], f32, tag="P")
                        for mt in (2 * mtp, 2 * mtp + 1):
                            dr_proj(
                                ps[:, (mt % 2) * LP : (mt % 2) * LP + LP],
                                wname,
                                pgboth,
                                w_sl=slice(mt * P, (mt + 1) * P),
                            )
                        out_ap = dst[:, 2 * mtp : 2 * mtp + 2, :].rearrange(
                            "p a b -> p (a b)"
                        )
                        with nc.allow_low_precision(**LOW):
                            evac(out_ap, ps)

                proj_lp("Wqp", qpT, lambda o, i: nc.vector.tensor_copy(out=o, in_=i))
                # qd+kd share one psum tile: qd cols 0:512, kd 512:1024
                _mark(nc, f"proj_qkd({b})")
                ps = pP.tile([P, 2 * LP], f32, tag="P")
                for qk, wname in enumerate(["Wqd", "Wkd"]):
                    for mt in range(KT):
                        dr_proj(
                            ps[:, qk * LP + mt * LD : qk * LP + (mt + 1) * LD],
                            wname,
                            dgboth,
                            w_sl=slice(mt * P, (mt + 1) * P),
                        )
                with nc.allow_low_precision(**LOW):
                    nc.scalar.copy(
                        out=qkdT.rearrange("p a b c -> p (a b c)"), in_=ps
                    )
                if tail is not None:
                    tail()
                proj_lp("Wkp", kpT, lambda o, i: nc.vector.tensor_copy(out=o, in_=i))

                # ---- logits + exp ----
                # E layout [p, lt, g, ld] with g = par*4 + hh <-> h = 2*hh+par
                E = apool.tile([P, KT, H, LD], bf16, tag="E")
                Pdp = apool.tile([P, H, LP], bf16, tag="Pdp")
                vp = apool.tile([P, KT, HID], bf16, tag="vp")
                vd = apool.tile([P, HID], bf16, tag="vd")

                Eh = apool.tile([P, KT, H, LD // 2], bf16, tag="Eh")
                Ph = apool.tile([P, H, LP // 2], bf16, tag="Ph")
                ADD = mybir.AluOpType.add

                def s_pd(lt):
                    _mark(nc, f"s_pd{lt}({b})")
                    ps = pS.tile([P, 2 * LP], f32, tag="S")
                    for par in range(2):
                        for hh in range(4):
                            h = 2 * hh + par
                            nc.tensor.matmul(
                                ps[:, par * LP + hh * LD : par * LP + (hh + 1) * LD],
                                lhsT=head_slice(qpT, h)[:, lt * P : (lt + 1) * P],
                                rhs=head_slice_qkd(1, h),
                                start=True,
                                stop=True,
                            )
                    nc.scalar.activation(
                        out=E[:, lt, :, :].rearrange("p a b -> p (a b)"),
                        in_=ps,
                        func=AF.Exp,
                        scale=EXP_SCALE,
                    )
                    with nc.allow_low_precision(**LOW):
                        nc.gpsimd.tensor_tensor(
                            out=Eh[:, lt],
                            in0=E[:, lt, :, 0 : LD // 2],
                            in1=E[:, lt, :, LD // 2 : LD],
                            op=ADD,
                        )

                def s_dp(i):
                    _mark(nc, f"s_dp{i}({b})")
                    ps = pS.tile([P, 2 * LP], f32, tag="S")
                    for par in range(2):
                        h = 2 * i + par
                        nc.tensor.matmul(
                            ps[:, par * LP : (par + 1) * LP],
                            lhsT=head_slice_qkd(0, h),
                            rhs=head_slice(kpT, h),
                            start=True,
                            stop=True,
                        )
                    nc.scalar.activation(
                        out=Pdp[:, 2 * i : 2 * i + 2, :].rearrange(
                            "p a b -> p (a b)"
                        ),
                        in_=ps,
                        func=AF.Exp,
                        scale=EXP_SCALE,
                    )
                    with nc.allow_low_precision(**LOW):
                        nc.gpsimd.tensor_tensor(
                            out=Ph[:, 2 * i : 2 * i + 2, :],
                            in0=Pdp[:, 2 * i : 2 * i + 2, 0 : LP // 2],
                            in1=Pdp[:, 2 * i : 2 * i + 2, LP // 2 : LP],
                            op=ADD,
                        )

                def vp_pair(mtp):
                    _mark(nc, f"vp{mtp}({b})")
                    ps = pP.tile([P, 2 * LP], f32, tag="P")
                    for mt in (2 * mtp, 2 * mtp + 1):
                        dr_proj(
                            ps[:, (mt % 2) * LP : (mt % 2) * LP + LP],
                            "Wvp",
                            pgboth,
                            lp_sl=slice(mt * P, (mt + 1) * P),
                            wlhs=False,
                        )
                    with nc.allow_low_precision(**LOW):
                        nc.scalar.copy(
                            out=vp[:, 2 * mtp : 2 * mtp + 2, :].rearrange(
                                "p a b -> p (a b)"
                            ),
                            in_=ps,
                        )

                def vd_calc():
                    _mark(nc, f"vd({b})")
                    ps = pP.tile([P, 2 * LP], f32, tag="P")
                    dr_proj(ps[:, 0:HID], "Wvd", dgboth, wlhs=False)
                    with nc.allow_low_precision(**LOW):
                        nc.scalar.copy(out=vd, in_=ps[:, 0:HID])

                s_pd(0)
                vp_pair(0)
                s_pd(1)
                if mid_pd is not None:
                    mid_pd()
                vp_pair(1)
                s_pd(2)
                vd_calc()
                s_pd(3)
                if grp_next is not None:
                    grp_next()
                s_dp(0)
                s_dp(1)
                if mid_dp is not None:
                    mid_dp()
                s_dp(2)
                s_dp(3)

                state[b] = dict(
                    E=E, Pdp=Pdp, Eh=Eh, Ph=Ph, wv=wv, vp=vp, vd=vd
                )

            def softtail_pd(b):
                _mark(nc, f"softtail({b})")
                st = state[b]
                Eh, wv = st["Eh"], st["wv"]
                Eq = apool.tile([P, KT, H, LD // 4], bf16, tag="Eq")
                with nc.allow_low_precision(**LOW):
                    nc.gpsimd.tensor_tensor(
                        out=Eq,
                        in0=Eh[:, :, :, 0 : LD // 4],
                        in1=Eh[:, :, :, LD // 4 : LD // 2],
                        op=mybir.AluOpType.add,
                    )
                rs_pd = apool.tile([P, KT, H], f32, tag="rs_pd")
                nc.vector.reduce_sum(out=rs_pd, in_=Eq, axis=AX.X)
                u_pd = apool.tile([P, KT, H], bf16, tag="u_pd")
                inv = apool.tile([P, KT, H], f32, tag="inv_pd")
                nc.vector.reciprocal(
                    out=inv.rearrange("p a b -> p (a b)"),
                    in_=rs_pd.rearrange("p a b -> p (a b)"),
                )
                for lt in range(KT):
                    nc.vector.tensor_scalar_mul(
                        u_pd[:, lt, :], inv[:, lt, :], wv[:, lt : lt + 1]
                    )
                st["u_pd"] = u_pd

            def softtail_dp(b):
                st = state[b]
                Ph, wv = st["Ph"], st["wv"]
                rs_dp = apool.tile([P, H], f32, tag="rs_dp")
                nc.vector.reduce_sum(out=rs_dp, in_=Ph, axis=AX.X)
                u_dp = apool.tile([P, H], bf16, tag="u_dp")
                inv2 = apool.tile([P, H], f32, tag="inv_dp")
                nc.vector.reciprocal(out=inv2, in_=rs_dp)
                nc.vector.tensor_scalar_mul(u_dp, inv2, wv[:, 4:5])
                st["u_dp"] = u_dp

            def finish_pd(b):
                _mark(nc, f"finish({b})")
                st = state[b]
                E, vd, u_pd = st["E"], st["vd"], st["u_pd"]
                ps = pS.tile([P, 2 * LP], f32, tag="S")
                # c_pd [ld, g] cols 0:8; contraction over lp (E partitions)
                for g in range(H):
                    for lt in range(KT):
                        nc.tensor.matmul(
                            ps[:, g : g + 1],
                            lhsT=E[:, lt, g, :],
                            rhs=u_pd[:, lt, g : g + 1],
                            start=(lt == 0),
                            stop=(lt == KT - 1),
                        )
                c_p = apool.tile([P, H], bf16, tag="c_p")
                with nc.allow_low_precision(**LOW):
                    nc.scalar.copy(out=c_p, in_=ps[:, 0:H])
                outst = apool.tile([1, 2 * HID], f32, tag="outst")
                st["outst"] = outst
                for g in range(H):
                    h = 2 * (g % 4) + g // 4
                    nc.tensor.matmul(
                        ps[0:1, HID + h * DH : HID + (h + 1) * DH],
                        lhsT=c_p[:, g : g + 1],
                        rhs=vd[:, h * DH : (h + 1) * DH],
                        start=True,
                        stop=True,
                    )
                nc.scalar.activation(
                    out=outst[:, 0:HID],
                    in_=ps[0:1, HID : 2 * HID],
                    func=AF.Copy,
                    scale=OUT_SCALE,
                )
                nc.sync.dma_start(out=out_d[b : b + 1, 0:HID], in_=outst[:, 0:HID])

            def finish_dp(b):
                st = state[b]
                Pdp, vp, u_dp, outst = st["Pdp"], st["vp"], st["u_dp"], st["outst"]
                ps = pS.tile([P, 2 * LP], f32, tag="S")
                # c_dp [lp_chunk, (lt, h)] cols 0:32
                for h in range(H):
                    for lt in range(KT):
                        nc.tensor.matmul(
                            ps[:, lt * H + h : lt * H + h + 1],
                            lhsT=Pdp[:, h, lt * P : (lt + 1) * P],
                            rhs=u_dp[:, h : h + 1],
                            start=True,
                            stop=True,
                        )
                c_d = apool.tile([P, KT * H], bf16, tag="c_d")
                with nc.allow_low_precision(**LOW):
                    nc.scalar.copy(out=c_d, in_=ps[:, 0 : KT * H])
                for h in range(H):
                    for lt in range(KT):
                        nc.tensor.matmul(
                            ps[0:1, HID + h * DH : HID + (h + 1) * DH],
                            lhsT=c_d[:, lt * H + h : lt * H + h + 1],
                            rhs=vp[:, lt, h * DH : (h + 1) * DH],
                            start=(lt == 0),
                            stop=(lt == KT - 1),
                        )
                nc.scalar.activation(
                    out=outst[:, HID : 2 * HID],
                    in_=ps[0:1, HID : 2 * HID],
                    func=AF.Copy,
                    scale=OUT_SCALE,
                )
                nc.sync.dma_start(
                    out=out_d[b : b + 1, HID : 2 * HID], in_=outst[:, HID : 2 * HID]
                )

            # ---- pipeline: grouping runs one batch ahead ----
            loads = [None] * (B + 2)
            loads[0] = issue_loads(0, first=True)
            loads[1] = issue_loads(1)
            grouping(0, loads[0])
            for b in range(B):
                if b + 2 < B:
                    loads[b + 2] = issue_loads(b + 2)
                prev = b - 1 if b > 0 else None
                compute(
                    b,
                    mid_pd=(lambda bb: lambda: finish_pd(bb))(prev)
                    if prev is not None
                    else None,
                    mid_dp=(lambda bb: lambda: finish_dp(bb))(prev)
                    if prev is not None
                    else None,
                    tail=(
                        lambda bb: lambda: (softtail_pd(bb), softtail_dp(bb))
                    )(prev)
                    if prev is not None
                    else None,
                    grp_next=(lambda bb, ld: lambda: grouping(bb, ld))(
                        b + 1, loads[b + 1] if b + 1 < B else None
                    )
                    if b + 1 < B
                    else None,
                )
            softtail_pd(B - 1)
            softtail_dp(B - 1)
            finish_pd(B - 1)
            finish_dp(B - 1)

    _split_excess_waits(nc)
    return nc


def _prep_in_maps(inputs):
    """Returns (in_maps, None) for the device path, or (None, fallback_out)."""
    protein = np.asarray(inputs["protein"], dtype=np.float32)
    drug = np.asarray(inputs["drug"], dtype=np.float32)
    mask_prot = np.asarray(inputs["mask_prot"]).astype(bool)
    mask_drug = np.asarray(inputs["mask_drug"]).astype(bool)
    Ws = {w: np.asarray(inputs[w], dtype=np.float32) for w in
          ["Wqp", "Wkp", "Wvp", "Wqd", "Wkd", "Wvd"]}

    import ml_dtypes

    bf = ml_dtypes.bfloat16
    f8 = ml_dtypes.float8_e4m3fn

    mp = mask_prot.reshape(NB, LP, GS_P).any(axis=2)
    md = mask_drug.reshape(NB, LD, GS_D).any(axis=2)
    if not (mp.all() and md.all()):
        return None, _numpy_reference(
            protein, drug, mask_prot, mask_drug,
            Ws["Wqp"], Ws["Wkp"], Ws["Wvp"], Ws["Wqd"], Ws["Wkd"], Ws["Wvd"],
        )
    wp = (mp.astype(np.float32) / mp.sum(axis=1, keepdims=True)).astype(np.float32)
    wd = (md.astype(np.float32) / md.sum(axis=1, keepdims=True)).astype(np.float32)

    prot_bf = protein.astype(bf)
    drug_bf = drug.astype(bf)

    # Wall8[p, wi*8 + kt*2 + s, o] = split_s(SC * W[wi][o, kt*128+p])
    wall8 = np.empty((P, 6 * 2 * KT, HID), dtype=f8)
    for wi, w in enumerate(["Wqp", "Wkp", "Wvp", "Wqd", "Wkd", "Wvd"]):
        wT = (Ws[w].T * SC).astype(np.float32)  # [d, o]
        hi = wT.astype(f8)
        lo = (wT - hi.astype(np.float32)).astype(f8)
        for kt in range(KT):
            wall8[:, wi * 2 * KT + 2 * kt, :] = hi[kt * P : (kt + 1) * P, :]
            wall8[:, wi * 2 * KT + 2 * kt + 1, :] = lo[kt * P : (kt + 1) * P, :]

    gboth = np.zeros((P, 96), dtype=bf)
    for g in range(P // GS_P):
        gboth[GS_P * g : GS_P * (g + 1), g] = SC / GS_P
    for g in range(P // GS_D):
        gboth[GS_D * g : GS_D * (g + 1), 32 + g] = SC / GS_D

    wvec = np.empty((NB, P, 5), dtype=np.float32)
    wvec[:, :, 0:4] = wp.reshape(NB, KT, P).transpose(0, 2, 1)
    wvec[:, :, 4] = wd

    in_maps = []
    for c in range(NCORES):
        sl = slice(c * B, (c + 1) * B)
        in_maps.append(
            {
                "protein": np.ascontiguousarray(prot_bf[sl]),
                "drug": np.ascontiguousarray(drug_bf[sl]),
                "Wall8": wall8,
                "Gboth": gboth,
                "wvec": np.ascontiguousarray(wvec[sl]),
            }
        )
    return in_maps, None


def kernel(**inputs):
    in_maps, fallback = _prep_in_maps(inputs)
    if in_maps is None:
        return fallback

    if "nc" not in _CACHE:
        _CACHE["nc"] = _build_nc()
    nc = _CACHE["nc"]

    from concourse.bass_utils import run_bass_kernel_spmd

    res = run_bass_kernel_spmd(nc, in_maps, list(range(NCORES)))
    _CACHE["last_results"] = res
    out = np.concatenate([res.results[c]["out"] for c in range(NCORES)], axis=0)
    return out.astype(np.float32)


if __name__ == "__main__":
    rng = np.random.default_rng(0)
    inputs = {
        "protein": rng.standard_normal((NB, LP_FULL, HID), dtype=np.float32),
        "drug": rng.standard_normal((NB, LD_FULL, HID), dtype=np.float32),
        "mask_prot": np.ones((NB, LP_FULL), dtype=bool),
        "mask_drug": np.ones((NB, LD_FULL), dtype=bool),
    }
    for w in ["Wqp", "Wkp", "Wvp", "Wqd", "Wkd", "Wvd"]:
        inputs[w] = rng.standard_normal((HID, HID), dtype=np.float32) / np.sqrt(HID)
    out = kernel(**inputs)
    ref = _numpy_reference(
        inputs["protein"], inputs["drug"], inputs["mask_prot"], inputs["mask_drug"],
        inputs["Wqp"], inputs["Wkp"], inputs["Wvp"],
        inputs["Wqd"], inputs["Wkd"], inputs["Wvd"],
    )
    err = np.abs(out - ref).max() / np.abs(ref).max()
    print("rel err:", err)


# revision 33
# speedup vs baseline: 1.2383x; 1.0153x over previous
"""Trainium2 Bass kernel for nn_CrossLayer (protein/drug cross-attention).

Reference math (per batch n):
  pg = group_mean(protein, 4)   # (512, 512)
  dg = group_mean(drug, 2)      # (128, 512)
  q/k/v projections (8 heads, dh=64), cross logits, softmax over the
  "other" sequence, attention-weighted values, masked mean-pool over the
  own sequence, concat(prot_embed, drug_embed) -> (1024,)

Key algebraic simplification: the pooled output
  prot_embed_h = (u_h.T @ P_h) @ vd_h   with u_h = w / rowsum_h
so the full (L x L') attention-output einsum is never materialized.

v3 scheme:
- All six projections run as fp8e4 DoubleRow matmuls with 3-term
  error compensation: x@W ~= xh@Wh + (xl@Wh + xh@Wl), where xh/xl are
  the fp8 hi/lo split of 64*x (scale keeps the lo part out of fp8
  subnormals). 6 DR instructions per 128-row output tile instead of 8
  bf16 instructions -> 25% fewer PE cycles, accuracy ~2x better than
  bf16.
- Scales ride the psum: q/k/v tensors are stored as 4096*value; the
  softmax descales via exp(scale=2^-24); outputs descale at the final
  copy (2^-12). All other PSUM evacuations are plain copies.
- Grouping is pipelined one batch ahead so its evac latency hides
  behind the previous batch's projections.
- Softmax: one 2-bank PSUM tile + one exp per 4 logit column-groups;
  rowsums via single DVE 4x-mode reduces; no Pool (no PSUM port).

Sharding: data-parallel over batch N=64 across 8 cores, weights
replicated.
"""

import sys

import numpy as np

for _p in ("/opt/trn_rl_repo", "/root/.axon_site/_ro/trn_rl_repo"):
    if _p not in sys.path:
        sys.path.insert(0, _p)

HID = 512
H = 8
DH = 64
GS_P = 4
GS_D = 2
LP_FULL = 2048
LD_FULL = 256
LP = LP_FULL // GS_P  # 512 grouped protein length
LD = LD_FULL // GS_D  # 128 grouped drug length
NB = 64  # total batch
NCORES = 8
B = NB // NCORES  # 8 batches per core
P = 128  # partitions
KT = HID // P  # 4 contraction tiles over hidden dim
SC = 64.0  # fp8 hi/lo split scale (pg and W each carry 64x)

_CACHE = {}
PHASE_MARKS = []  # (label, first_I_number) appended during _build_nc


def _mark(nc, label):
    nm = nc.get_next_instruction_name()  # consumes one name: I-<n>
    PHASE_MARKS.append((label, int(nm.split("-")[1])))


def _numpy_reference(protein, drug, mask_prot, mask_drug, Wqp, Wkp, Wvp, Wqd, Wkd, Wvd):
    """Exact reference math in numpy (fallback for non-trivial masks)."""
    INF = 1000000.0

    def group(x, m, gs):
        n, l, d = x.shape
        xg = x.reshape(n, l // gs, gs, d).mean(axis=2)
        mg = m.reshape(n, l // gs, gs).any(axis=2)
        return xg, mg

    def heads(x):
        n, l, d = x.shape
        return x.reshape(n, l, H, d // H)

    pg, mp = group(protein, mask_prot, GS_P)
    dg, md = group(drug, mask_drug, GS_D)
    qp = heads(pg @ Wqp.T)
    kp = heads(pg @ Wkp.T)
    vp = heads(pg @ Wvp.T)
    qd = heads(dg @ Wqd.T)
    kd = heads(dg @ Wkd.T)
    vd = heads(dg @ Wvd.T)

    def alpha(logits, mr, mc):
        pair = mr[:, :, None, None] & mc[:, None, :, None]
        logits = np.where(pair, logits, logits - INF)
        m = logits.max(axis=2, keepdims=True)
        e = np.exp(logits - m)
        a = e / e.sum(axis=2, keepdims=True)
        return np.where(mr[:, :, None, None], a, 0.0)

    lpd = np.einsum("blhd,bkhd->blkh", qp, kd)
    ldp = np.einsum("blhd,bkhd->blkh", qd, kp)
    apd = alpha(lpd, mp, md)
    adp = alpha(ldp, md, mp)
    n = pg.shape[0]
    pe = np.einsum("blkh,bkhd->blhd", apd, vd).reshape(n, pg.shape[1], -1)
    de = np.einsum("blkh,bkhd->blhd", adp, vp).reshape(n, dg.shape[1], -1)
    mpf = mp.astype(pe.dtype)
    mdf = md.astype(de.dtype)
    pemb = (pe * mpf[:, :, None]).sum(axis=1) / mpf.sum(axis=-1)[:, None]
    demb = (de * mdf[:, :, None]).sum(axis=1) / mdf.sum(axis=-1)[:, None]
    return np.concatenate([pemb, demb], axis=1).astype(np.float32)


def _split_excess_waits(nc):
    """Split multi-sem waits into single-wait engine NOPs.

    TPB compute-instruction encodings carry exactly one sync-wait slot;
    Tile sometimes assigns 2-3 waits to one instruction, which walrus
    rejects. Prefixing the instruction with NOPs that each carry one of
    the excess waits is semantically identical (engines dispatch their
    stream in order). DMA waits instead chain through SP NOPs bumping a
    gate semaphore (DGE wait conditions fire autonomously).
    """
    import concourse.mybir as mybir
    import bass_rust

    MULTI_OK = {"InstEventSemaphore"}

    def make_nop(engine):
        eng = {
            mybir.EngineType.PE: nc.tensor,
            mybir.EngineType.Activation: nc.scalar,
            mybir.EngineType.DVE: nc.vector,
            mybir.EngineType.Pool: nc.gpsimd,
            mybir.EngineType.SP: nc.sync,
        }[engine]
        bi = eng.nop(nofuse=True)
        inst = bi.ins if hasattr(bi, "ins") else bi
        for bbw in nc.bb_map.values():
            lst = bbw.bb.instructions
            if lst and lst[-1] is inst:
                lst.pop()
                break
        return inst

    used = set()
    for bbw in nc.bb_map.values():
        for inst in bbw.bb.instructions:
            si = getattr(inst, "sync_info", None)
            if si is None:
                continue
            for w in si.on_wait or []:
                used.add(w.id)
            for u in si.on_update or []:
                used.add(u.id)
    gate_id = max(used) + 1 if used else 100
    assert gate_id < 250, f"no free semaphore for DMA gate ({gate_id})"
    gate_count = 0

    n_split = 0
    for bbw in list(nc.bb_map.values()):
        bb = bbw.bb
        lst = bb.instructions
        idx = 0
        while idx < len(lst):
            inst = lst[idx]
            si = getattr(inst, "sync_info", None)
            if (
                si is not None
                and si.on_wait
                and len(si.on_wait) > 1
                and type(inst).__name__ not in MULTI_OK
            ):
                waits = list(si.on_wait)
                if type(inst).__name__ == "InstDMACopy":
                    for w in waits:
                        nop = make_nop(mybir.EngineType.SP)
                        nop.sync_info = type(si)(on_wait=[w], on_update=[])
                        lst.insert(idx, nop)
                        idx += 1
                        n_split += 1
                    gate_count += 1
                    nop.sync_info = type(si)(
                        on_wait=[w],
                        on_update=[
                            bass_rust.SyncUpdate(
                                sync_type="semaphore",
                                id=gate_id,
                                ant_name=f"dma_gate_{gate_id}",
                                update_mode="sem-inc",
                                update_value=1,
                                update_reg=None,
                            )
                        ],
                    )
                    inst.sync_info = type(si)(
                        on_wait=[
                            bass_rust.SyncWait(
                                sync_type="semaphore",
                                id=gate_id,
                                ant_name=f"dma_gate_{gate_id}",
                                wait_mode="sem-ge-imm",
                                wait_value=gate_count,
                                wait_reg=None,
                            )
                        ],
                        on_update=si.on_update,
                    )
                else:
                    extra, keep = waits[:-1], waits[-1:]
                    for w in extra:
                        nop = make_nop(inst.engine)
                        nop.sync_info = type(si)(on_wait=[w], on_update=[])
                        lst.insert(idx, nop)
                        idx += 1
                        n_split += 1
                    inst.sync_info = type(si)(on_wait=keep, on_update=si.on_update)
            idx += 1
    return n_split


def _build_nc():
    import concourse.bass as bass
    import concourse.mybir as mybir
    import concourse.tile as tile

    bf16 = mybir.dt.bfloat16
    f32 = mybir.dt.float32
    fp8 = mybir.dt.float8e4
    DR = mybir.MatmulPerfMode.DoubleRow
    AF = mybir.ActivationFunctionType
    AX = mybir.AxisListType
    SUB = mybir.AluOpType.subtract

    nc = bass.Bass()

    prot = nc.declare_dram_parameter("protein", [B, LP_FULL, HID], bf16, isOutput=False)
    drug = nc.declare_dram_parameter("drug", [B, LD_FULL, HID], bf16, isOutput=False)
    # fp8 hi/lo split weights: rows r = wi*2*KT + kt*2 + s (s: 0=hi, 1=lo)
    # value = split_s(SC * W[wi][o, kt*128+p])
    w_d = nc.declare_dram_parameter("Wall8", [P, 6 * 2 * KT, HID], fp8, isOutput=False)
    # grouping matrices: cols [0:32) Gp*(SC/4), [32:96) Gd*(SC/2)
    g_d = nc.declare_dram_parameter("Gboth", [P, 96], bf16, isOutput=False)
    # pooling weights: [b, p, 0:4] = wp[b, t*128+p], [b, p, 4] = wd[b, p]
    wv_d = nc.declare_dram_parameter("wvec", [B, P, 5], f32, isOutput=False)
    out_d = nc.declare_dram_parameter("out", [B, 2 * HID], f32, isOutput=True)

    WI = {"Wqp": 0, "Wkp": 1, "Wvp": 2, "Wqd": 3, "Wkd": 4, "Wvd": 5}
    EXP_SCALE = 1.0 / (SC * SC * SC * SC)  # 2^-24: descale logits at exp
    OUT_SCALE = 1.0 / (SC * SC)  # 2^-12: descale v at output evac

    with tile.TileContext(nc) as tc:
        with (
            tc.tile_pool(name="const", bufs=1) as cpool,
            tc.tile_pool(name="pt", bufs=2) as ptpool,
            tc.tile_pool(name="act", bufs=2) as apool,
            tc.tile_pool(name="pP", bufs=2, space="PSUM") as pP,
            tc.tile_pool(name="pS", bufs=2, space="PSUM") as pS,
        ):
            g_sb = cpool.tile([P, 96], bf16, tag="g")
            w_sb = cpool.tile([P, 6 * 2 * KT, HID], fp8, tag="wall")
            gp_sb = g_sb[:, 0:32]
            gd_sb = g_sb[:, 32:96]

            def w8(wname, row, osl=slice(0, HID)):
                """Single [128, o] row of the hi/lo weight stack."""
                return w_sb[:, WI[wname] * 2 * KT + row, osl]

            def w8p(wname, r0, step, osl=slice(0, HID)):
                """[128, 2, o] row-pair (the two DR groups)."""
                base = WI[wname] * 2 * KT
                return w_sb[:, base + r0 : base + r0 + step + 1 : step, osl]

            state = [None] * B
            # pgboth rows: kt*2+0 = lo, kt*2+1 = hi (pairs with Wall8's hi,lo)
            grp = [None] * B  # (pgboth, dgboth, wv) per batch
            LOW = dict(reason="bf16/fp8 activations; tolerance is 2e-2")

            def issue_loads(b, first=False):
                pa = ptpool.tile([P, 8, HID], bf16, tag="pa")
                pb = ptpool.tile([P, 8, HID], bf16, tag="pb")
                dr = ptpool.tile([P, 2, HID], bf16, tag="dr")
                wv = ptpool.tile([P, 5], f32, tag="wv")
                prot_r = prot[b].rearrange("(t p) d -> p t d", p=P)
                if first:
                    nc.sync.dma_start(out=g_sb, in_=g_d[:, :])
                nc.sync.dma_start(out=wv, in_=wv_d[b])
                nc.sync.dma_start(out=pa, in_=prot_r[:, 0:8, :])
                nc.sync.dma_start(out=pb, in_=prot_r[:, 8:16, :])
                nc.sync.dma_start(
                    out=dr, in_=drug[b].rearrange("(t p) d -> p t d", p=P)
                )
                if first:
                    # weights are first needed by the projections, after
                    # grouping(0) -- load them behind batch-0 inputs
                    nc.sync.dma_start(out=w_sb[:, 0:24, :], in_=w_d[:, 0:24, :])
                    nc.sync.dma_start(out=w_sb[:, 24:48, :], in_=w_d[:, 24:48, :])
                return pa, pb, dr, wv

            def grouping(b, loads):
                """Fill pgboth/dgboth (fp8 hi/lo, values SC*pg) for batch b."""
                _mark(nc, f"grouping({b})")
                pa, pb, dr, wv = loads
                pgboth = apool.tile([P, 2 * KT, LP], fp8, tag="pgboth")
                dgboth = apool.tile([P, 2 * KT, LD], fp8, tag="dgboth")
                for ktp in range(2):
                    ps = pP.tile([P, 2 * LP], f32, tag="P")
                    for t in range(16):
                        src = pa if t < 8 else pb
                        for kt in (2 * ktp, 2 * ktp + 1):
                            nc.tensor.matmul(
                                ps[:, (kt % 2) * LP + t * 32 : (kt % 2) * LP + t * 32 + 32],
                                lhsT=src[:, t % 8, kt * P : (kt + 1) * P],
                                rhs=gp_sb,
                                start=True,
                                stop=True,
                            )
                    # hi rows (4ktp+1, 4ktp+3), lo rows (4ktp, 4ktp+2)
                    hi = pgboth[:, 4 * ktp + 1 : 4 * ktp + 4 : 2, :]
                    lo = pgboth[:, 4 * ktp : 4 * ktp + 3 : 2, :]
                    ps3 = ps.rearrange("p (a b) -> p a b", a=2)
                    with nc.allow_low_precision(**LOW):
                        nc.vector.tensor_copy(out=hi, in_=ps3)
                        nc.vector.tensor_tensor(
                            out=lo, in0=ps3, in1=hi, op=SUB
                        )
                ps = pP.tile([P, 2 * LP], f32, tag="P")
                psd = ps[:, 0 : KT * LD]
                for kt in range(KT):
                    for t in range(2):
                        nc.tensor.matmul(
                            psd[:, kt * LD + t * 64 : kt * LD + (t + 1) * 64],
                            lhsT=dr[:, t, kt * P : (kt + 1) * P],
                            rhs=gd_sb,
                            start=True,
                            stop=True,
                        )
                psd3 = psd.rearrange("p (a b) -> p a b", a=KT)
                hi = dgboth[:, 1 : 2 * KT : 2, :]
                lo = dgboth[:, 0 : 2 * KT - 1 : 2, :]
                with nc.allow_low_precision(**LOW):
                    nc.vector.tensor_copy(out=hi, in_=psd3)
                    nc.vector.tensor_tensor(out=lo, in0=psd3, in1=hi, op=SUB)
                grp[b] = (pgboth, dgboth, wv)

            def dr_proj(ps_out, wname, both, lp_sl=None, w_sl=None, wlhs=True):
                """3-term compensated DR projection into ps_out.

                wlhs=True: lhsT = weight rows, rhs = activation rows
                           (out = [o_tile, l]).
                wlhs=False: lhsT = activation rows, rhs = weight rows
                           (out = [l_tile, o]).
                both rows: kt*2+0 = lo, kt*2+1 = hi.
                """
                n = 0

                def mm(wpair, apair, last):
                    nonlocal n
                    lhsT, rhs = (wpair, apair) if wlhs else (apair, wpair)
                    nc.tensor.matmul(
                        ps_out,
                        lhsT=lhsT,
                        rhs=rhs,
                        start=(n == 0),
                        stop=last,
                        perf_mode=DR,
                    )
                    n += 1

                asl = lp_sl if lp_sl is not None else slice(None)
                # HI terms: W rows (4i, 4i+2) [hi pair], act rows (4i+1, 4i+3)
                for i in range(2):
                    mm(
                        w8p(wname, 4 * i, 2, w_sl) if w_sl else w8p(wname, 4 * i, 2),
                        both[:, 4 * i + 1 : 4 * i + 4 : 2, asl],
                        False,
                    )
                # CORR: W rows (2k, 2k+1) = (hi_k, lo_k), act rows (2k, 2k+1)
                # = (lo_k, hi_k) -> lo@Whi + hi@Wlo
                for k in range(KT):
                    mm(
                        w8p(wname, 2 * k, 1, w_sl) if w_sl else w8p(wname, 2 * k, 1),
                        both[:, 2 * k : 2 * k + 2, asl],
                        k == KT - 1,
                    )

            def head_slice(tens, h):
                return tens[64 * (h % 2) : 64 * (h % 2) + 64, h // 2, :]

            proj = [None] * (B + 1)  # {"qpT":..., "qkdT":...} per batch

            def projhead_qp(b):
                pgboth, dgboth, wv = grp[b]
                _mark(nc, f"proj_Wqp({b})")
                qpT = apool.tile([P, KT, LP], bf16, tag="qpT")
                proj[b] = {"qpT": qpT}
                for mtp in range(2):
                    ps = pP.tile([P, 2 * LP], f32, tag="P")
                    for mt in (2 * mtp, 2 * mtp + 1):
                        dr_proj(
                            ps[:, (mt % 2) * LP : (mt % 2) * LP + LP],
                            "Wqp",
                            pgboth,
                            w_sl=slice(mt * P, (mt + 1) * P),
                        )
                    out_ap = qpT[:, 2 * mtp : 2 * mtp + 2, :].rearrange(
                        "p a b -> p (a b)"
                    )
                    with nc.allow_low_precision(**LOW):
                        nc.vector.tensor_copy(out=out_ap, in_=ps)

            def projhead_qkd(b):
                pgboth, dgboth, wv = grp[b]
                _mark(nc, f"proj_qkd({b})")
                qkdT = apool.tile([P, 2, KT, LD], bf16, tag="qkdT")
                proj[b]["qkdT"] = qkdT
                ps = pP.tile([P, 2 * LP], f32, tag="P")
                for qk, wname in enumerate(["Wqd", "Wkd"]):
                    for mt in range(KT):
                        dr_proj(
                            ps[:, qk * LP + mt * LD : qk * LP + (mt + 1) * LD],
                            wname,
                            dgboth,
                            w_sl=slice(mt * P, (mt + 1) * P),
                        )
                with nc.allow_low_precision(**LOW):
                    nc.scalar.copy(
                        out=qkdT.rearrange("p a b c -> p (a b c)"), in_=ps
                    )

            def compute(b, mid_pd=None, mid_dp=None, tail=None, grp_next=None,
                        nxt_qp=None, nxt_qkd=None, last=False, first=False):
                pgboth, dgboth, wv = grp[b]
                qpT = proj[b]["qpT"]
                qkdT = proj[b]["qkdT"]
                kpT = apool.tile([P, KT, LP], bf16, tag="kpT")

                def head_slice_qkd(qk, h):
                    return qkdT[64 * (h % 2) : 64 * (h % 2) + 64, qk, h // 2, :]
                # E layout [p, lt, g, ld] with g = par*4 + hh <-> h = 2*hh+par
                E = apool.tile([P, KT, H, LD], bf16, tag="E")
                Pdp = apool.tile([P, H, LP], bf16, tag="Pdp")
                vp = apool.tile([P, KT, HID], bf16, tag="vp")
                vd = apool.tile([P, HID], bf16, tag="vd")

                def vd_calc():
                    _mark(nc, f"vd({b})")
                    ps = pP.tile([P, 2 * LP], f32, tag="P")
                    dr_proj(ps[:, 0:HID], "Wvd", dgboth, wlhs=False)
                    with nc.allow_low_precision(**LOW):
                        nc.scalar.copy(out=vd, in_=ps[:, 0:HID])

                def proj_lp(wname, dst, evac):
                    _mark(nc, f"proj_{wname}({b})")
                    for mtp in range(2):
                        ps = pP.tile([P, 2 * LP], f32, tag="P")
                        for mt in (2 * mtp, 2 * mtp + 1):
                            dr_proj(
                                ps[:, (mt % 2) * LP : (mt % 2) * LP + LP],
                                wname,
                                pgboth,
                                w_sl=slice(mt * P, (mt + 1) * P),
                            )
                        out_ap = dst[:, 2 * mtp : 2 * mtp + 2, :].rearrange(
                            "p a b -> p (a b)"
                        )
                        with nc.allow_low_precision(**LOW):
                            evac(out_ap, ps)




                Eh = apool.tile([P, KT, H, LD // 2], bf16, tag="Eh")
                Ph = apool.tile([P, H, LP // 2], bf16, tag="Ph")
                ADD = mybir.AluOpType.add
                state[b] = dict(
                    E=E, Pdp=Pdp, Eh=Eh, Ph=Ph, wv=wv, vp=vp, vd=vd
                )
                if last:
                    outst_l = apool.tile([1, 2 * HID], f32, tag="outst")
                    state[b]["outst"] = outst_l

                def s_pd(lt):
                    _mark(nc, f"s_pd{lt}({b})")
                    ps = pS.tile([P, 2 * LP], f32, tag="S")
                    for par in range(2):
                        for hh in range(4):
                            h = 2 * hh + par
                            nc.tensor.matmul(
                                ps[:, par * LP + hh * LD : par * LP + (hh + 1) * LD],
                                lhsT=head_slice(qpT, h)[:, lt * P : (lt + 1) * P],
                                rhs=head_slice_qkd(1, h),
                                start=True,
                                stop=True,
                            )
                    nc.scalar.activation(
                        out=E[:, lt, :, :].rearrange("p a b -> p (a b)"),
                        in_=ps,
                        func=AF.Exp,
                        scale=EXP_SCALE,
                    )
                    with nc.allow_low_precision(**LOW):
                        nc.gpsimd.tensor_tensor(
                            out=Eh[:, lt],
                            in0=E[:, lt, :, 0 : LD // 2],
                            in1=E[:, lt, :, LD // 2 : LD],
                            op=ADD,
                        )

                def s_dp(i):
                    _mark(nc, f"s_dp{i}({b})")
                    ps = pS.tile([P, 2 * LP], f32, tag="S")
                    for par in range(2):
                        h = 2 * i + par
                        nc.tensor.matmul(
                            ps[:, par * LP : (par + 1) * LP],
                            lhsT=head_slice_qkd(0, h),
                            rhs=head_slice(kpT, h),
                            start=True,
                            stop=True,
                        )
                    nc.scalar.activation(
                        out=Pdp[:, 2 * i : 2 * i + 2, :].rearrange(
                            "p a b -> p (a b)"
                        ),
                        in_=ps,
                        func=AF.Exp,
                        scale=EXP_SCALE,
                    )
                    with nc.allow_low_precision(**LOW):
                        nc.gpsimd.tensor_tensor(
                            out=Ph[:, 2 * i : 2 * i + 2, :],
                            in0=Pdp[:, 2 * i : 2 * i + 2, 0 : LP // 2],
                            in1=Pdp[:, 2 * i : 2 * i + 2, LP // 2 : LP],
                            op=ADD,
                        )

                def vp_pair(mtp):
                    _mark(nc, f"vp{mtp}({b})")
                    ps = pP.tile([P, 2 * LP], f32, tag="P")
                    for mt in (2 * mtp, 2 * mtp + 1):
                        dr_proj(
                            ps[:, (mt % 2) * LP : (mt % 2) * LP + LP],
                            "Wvp",
                            pgboth,
                            lp_sl=slice(mt * P, (mt + 1) * P),
                            wlhs=False,
                        )
                    with nc.allow_low_precision(**LOW):
                        nc.scalar.copy(
                            out=vp[:, 2 * mtp : 2 * mtp + 2, :].rearrange(
                                "p a b -> p (a b)"
                            ),
                            in_=ps,
                        )

                import os as _os

                _order = _os.environ.get(
                    "KSCHED",
                    "tail,p0,kp,v0,p1,fpd,v1,p2,vd,grp,p3,d0,d1,fdp,d2,d3,nqp,nqkd",
                ).split(",")
                _ph = {
                    "kp": lambda: proj_lp(
                        "Wkp", kpT, lambda o, i: nc.vector.tensor_copy(out=o, in_=i)
                    ),
                    "nqp": nxt_qp if nxt_qp is not None else (lambda: None),
                    "nqkd": nxt_qkd if nxt_qkd is not None else (lambda: None),
                    "tail": tail if tail is not None else (lambda: None),
                    "p0": lambda: s_pd(0),
                    "p1": lambda: s_pd(1),
                    "p2": lambda: s_pd(2),
                    "p3": lambda: s_pd(3),
                    "v0": lambda: vp_pair(0),
                    "v1": lambda: vp_pair(1),
                    "vd": vd_calc,
                    "fpd": mid_pd if mid_pd is not None else (lambda: None),
                    "fdp": mid_dp if mid_dp is not None else (lambda: None),
                    "grp": grp_next if grp_next is not None else (lambda: None),
                    "d0": lambda: s_dp(0),
                    "d1": lambda: s_dp(1),
                    "d2": lambda: s_dp(2),
                    "d3": lambda: s_dp(3),
                }
                for _p in _order:
                    _ph[_p]()
                if last:
                    softtail_pd(b)
                    softtail_dp(b)
                    finish_pd(b)
                    finish_dp(b)


            def softtail_pd(b):
                _mark(nc, f"softtail({b})")
                st = state[b]
                Eh, wv = st["Eh"], st["wv"]
                Eq = apool.tile([P, KT, H, LD // 4], bf16, tag="Eq")
                with nc.allow_low_precision(**LOW):
                    nc.gpsimd.tensor_tensor(
                        out=Eq,
                        in0=Eh[:, :, :, 0 : LD // 4],
                        in1=Eh[:, :, :, LD // 4 : LD // 2],
                        op=mybir.AluOpType.add,
                    )
                rs_pd = apool.tile([P, KT, H], f32, tag="rs_pd")
                nc.vector.reduce_sum(out=rs_pd, in_=Eq, axis=AX.X)
                u_pd = apool.tile([P, KT, H], bf16, tag="u_pd")
                inv = apool.tile([P, KT, H], f32, tag="inv_pd")
                nc.vector.reciprocal(
                    out=inv.rearrange("p a b -> p (a b)"),
                    in_=rs_pd.rearrange("p a b -> p (a b)"),
                )
                for lt in range(KT):
                    nc.vector.tensor_scalar_mul(
                        u_pd[:, lt, :], inv[:, lt, :], wv[:, lt : lt + 1]
                    )
                st["u_pd"] = u_pd

            def softtail_dp(b, deep=False):
                st = state[b]
                Ph, wv = st["Ph"], st["wv"]
                rs_dp = apool.tile([P, H], f32, tag="rs_dp")
                if deep:
                    Pq = apool.tile([P, H, LP // 4], bf16, tag="Pq")
                    with nc.allow_low_precision(**LOW):
                        nc.gpsimd.tensor_tensor(
                            out=Pq,
                            in0=Ph[:, :, 0 : LP // 4],
                            in1=Ph[:, :, LP // 4 : LP // 2],
                            op=mybir.AluOpType.add,
                        )
                    nc.vector.reduce_sum(out=rs_dp, in_=Pq, axis=AX.X)
                else:
                    nc.vector.reduce_sum(out=rs_dp, in_=Ph, axis=AX.X)
                u_dp = apool.tile([P, H], bf16, tag="u_dp")
                inv2 = apool.tile([P, H], f32, tag="inv_dp")
                nc.vector.reciprocal(out=inv2, in_=rs_dp)
                nc.vector.tensor_scalar_mul(u_dp, inv2, wv[:, 4:5])
                st["u_dp"] = u_dp

            def finish_pd(b):
                _mark(nc, f"finish({b})")
                st = state[b]
                E, vd, u_pd = st["E"], st["vd"], st["u_pd"]
                ps = pS.tile([P, 2 * LP], f32, tag="S")
                # c_pd [ld, g] cols 0:8; contraction over lp (E partitions)
                for g in range(H):
                    for lt in range(KT):
                        nc.tensor.matmul(
                            ps[:, g : g + 1],
                            lhsT=E[:, lt, g, :],
                            rhs=u_pd[:, lt, g : g + 1],
                            start=(lt == 0),
                            stop=(lt == KT - 1),
                        )
                c_p = apool.tile([P, H], bf16, tag="c_p")
                with nc.allow_low_precision(**LOW):
                    nc.scalar.copy(out=c_p, in_=ps[:, 0:H])
                outst = st.get("outst")
                if outst is None:
                    outst = apool.tile([1, 2 * HID], f32, tag="outst")
                    st["outst"] = outst
                for g in range(H):
                    h = 2 * (g % 4) + g // 4
                    nc.tensor.matmul(
                        ps[0:1, HID + h * DH : HID + (h + 1) * DH],
                        lhsT=c_p[:, g : g + 1],
                        rhs=vd[:, h * DH : (h + 1) * DH],
                        start=True,
                        stop=True,
                    )
                nc.scalar.activation(
                    out=outst[:, 0:HID],
                    in_=ps[0:1, HID : 2 * HID],
                    func=AF.Copy,
                    scale=OUT_SCALE,
                )
                nc.sync.dma_start(out=out_d[b : b + 1, 0:HID], in_=outst[:, 0:HID])

            def finish_dp(b):
                st = state[b]
                Pdp, vp, u_dp, outst = st["Pdp"], st["vp"], st["u_dp"], st["outst"]
                ps = pS.tile([P, 2 * LP], f32, tag="S")
                # c_dp [lp_chunk, (lt, h)] cols 0:32
                for h in range(H):
                    for lt in range(KT):
                        nc.tensor.matmul(
                            ps[:, lt * H + h : lt * H + h + 1],
                            lhsT=Pdp[:, h, lt * P : (lt + 1) * P],
                            rhs=u_dp[:, h : h + 1],
                            start=True,
                            stop=True,
                        )
                c_d = apool.tile([P, KT * H], bf16, tag="c_d")
                with nc.allow_low_precision(**LOW):
                    nc.scalar.copy(out=c_d, in_=ps[:, 0 : KT * H])
                for h in range(H):
                    for lt in range(KT):
                        nc.tensor.matmul(
                            ps[0:1, HID + h * DH : HID + (h + 1) * DH],
                            lhsT=c_d[:, lt * H + h : lt * H + h + 1],
                            rhs=vp[:, lt, h * DH : (h + 1) * DH],
                            start=(lt == 0),
                            stop=(lt == KT - 1),
                        )
                nc.scalar.activation(
                    out=outst[:, HID : 2 * HID],
                    in_=ps[0:1, HID : 2 * HID],
                    func=AF.Copy,
                    scale=OUT_SCALE,
                )
                nc.sync.dma_start(
                    out=out_d[b : b + 1, HID : 2 * HID], in_=outst[:, HID : 2 * HID]
                )

            # ---- pipeline: grouping runs one batch ahead ----
            loads = [None] * (B + 2)
            loads[0] = issue_loads(0, first=True)
            loads[1] = issue_loads(1)
            grouping(0, loads[0])
            projhead_qp(0)
            projhead_qkd(0)
            for b in range(B):
                if b + 2 < B:
                    loads[b + 2] = issue_loads(b + 2)
                prev = b - 1 if b > 0 else None
                compute(
                    b,
                    mid_pd=(lambda bb: lambda: finish_pd(bb))(prev)
                    if prev is not None
                    else None,
                    mid_dp=(lambda bb: lambda: finish_dp(bb))(prev)
                    if prev is not None
                    else None,
                    tail=(
                        lambda bb: lambda: (softtail_pd(bb), softtail_dp(bb))
                    )(prev)
                    if prev is not None
                    else None,
                    grp_next=(lambda bb, ld: lambda: grouping(bb, ld))(
                        b + 1, loads[b + 1] if b + 1 < B else None
                    )
                    if b + 1 < B
                    else None,
                    nxt_qp=(lambda bb: lambda: projhead_qp(bb))(b + 1)
                    if b + 1 < B
                    else None,
                    nxt_qkd=(lambda bb: lambda: projhead_qkd(bb))(b + 1)
                    if b + 1 < B
                    else None,
                    last=(b == B - 1),
                    first=(b == 0),
                )

    _split_excess_waits(nc)
    return nc


def _prep_in_maps(inputs):
    """Returns (in_maps, None) for the device path, or (None, fallback_out)."""
    protein = np.asarray(inputs["protein"], dtype=np.float32)
    drug = np.asarray(inputs["drug"], dtype=np.float32)
    mask_prot = np.asarray(inputs["mask_prot"]).astype(bool)
    mask_drug = np.asarray(inputs["mask_drug"]).astype(bool)
    Ws = {w: np.asarray(inputs[w], dtype=np.float32) for w in
          ["Wqp", "Wkp", "Wvp", "Wqd", "Wkd", "Wvd"]}

    import ml_dtypes

    bf = ml_dtypes.bfloat16
    f8 = ml_dtypes.float8_e4m3fn

    mp = mask_prot.reshape(NB, LP, GS_P).any(axis=2)
    md = mask_drug.reshape(NB, LD, GS_D).any(axis=2)
    if not (mp.all() and md.all()):
        return None, _numpy_reference(
            protein, drug, mask_prot, mask_drug,
            Ws["Wqp"], Ws["Wkp"], Ws["Wvp"], Ws["Wqd"], Ws["Wkd"], Ws["Wvd"],
        )
    wp = (mp.astype(np.float32) / mp.sum(axis=1, keepdims=True)).astype(np.float32)
    wd = (md.astype(np.float32) / md.sum(axis=1, keepdims=True)).astype(np.float32)

    prot_bf = protein.astype(bf)
    drug_bf = drug.astype(bf)

    # Wall8[p, wi*8 + kt*2 + s, o] = split_s(SC * W[wi][o, kt*128+p])
    wall8 = np.empty((P, 6 * 2 * KT, HID), dtype=f8)
    for wi, w in enumerate(["Wqp", "Wkp", "Wvp", "Wqd", "Wkd", "Wvd"]):
        wT = (Ws[w].T * SC).astype(np.float32)  # [d, o]
        hi = wT.astype(f8)
        lo = (wT - hi.astype(np.float32)).astype(f8)
        for kt in range(KT):
            wall8[:, wi * 2 * KT + 2 * kt, :] = hi[kt * P : (kt + 1) * P, :]
            wall8[:, wi * 2 * KT + 2 * kt + 1, :] = lo[kt * P : (kt + 1) * P, :]

    gboth = np.zeros((P, 96), dtype=bf)
    for g in range(P // GS_P):
        gboth[GS_P * g : GS_P * (g + 1), g] = SC / GS_P
    for g in range(P // GS_D):
        gboth[GS_D * g : GS_D * (g + 1), 32 + g] = SC / GS_D

    wvec = np.empty((NB, P, 5), dtype=np.float32)
    wvec[:, :, 0:4] = wp.reshape(NB, KT, P).transpose(0, 2, 1)
    wvec[:, :, 4] = wd

    in_maps = []
    for c in range(NCORES):
        sl = slice(c * B, (c + 1) * B)
        in_maps.append(
            {
                "protein": np.ascontiguousarray(prot_bf[sl]),
                "drug": np.ascontiguousarray(drug_bf[sl]),
                "Wall8": wall8,
                "Gboth": gboth,
                "wvec": np.ascontiguousarray(wvec[sl]),
            }
        )
    return in_maps, None


def kernel(**inputs):
    in_maps, fallback = _prep_in_maps(inputs)
    if in_maps is None:
        return fallback

    if "nc" not in _CACHE:
        _CACHE["nc"] = _build_nc()
    nc = _CACHE["nc"]

    from concourse.bass_utils import run_bass_kernel_spmd

    res = run_bass_kernel_spmd(nc, in_maps, list(range(NCORES)))
    _CACHE["last_results"] = res
    out = np.concatenate([res.results[c]["out"] for c in range(NCORES)], axis=0)
    return out.astype(np.float32)


if __name__ == "__main__":
    rng = np.random.default_rng(0)
    inputs = {
        "protein": rng.standard_normal((NB, LP_FULL, HID), dtype=np.float32),
        "drug": rng.standard_normal((NB, LD_FULL, HID), dtype=np.float32),
        "mask_prot": np.ones((NB, LP_FULL), dtype=bool),
        "mask_drug": np.ones((NB, LD_FULL), dtype=bool),
    }
    for w in ["Wqp", "Wkp", "Wvp", "Wqd", "Wkd", "Wvd"]:
        inputs[w] = rng.standard_normal((HID, HID), dtype=np.float32) / np.sqrt(HID)
    out = kernel(**inputs)
    ref = _numpy_reference(
        inputs["protein"], inputs["drug"], inputs["mask_prot"], inputs["mask_drug"],
        inputs["Wqp"], inputs["Wkp"], inputs["Wvp"],
        inputs["Wqd"], inputs["Wkd"], inputs["Wvd"],
    )
    err = np.abs(out - ref).max() / np.abs(ref).max()
    print("rel err:", err)
